# revision 19
# baseline (speedup 1.0000x reference)
"""Trainium2 Bass kernel for Cylinder3D point-pillar feature net.

Pipeline (reference semantics):
  h = BN0(pt_fea); h = relu(BN1(h@w1+b1)); h = relu(BN2(h@w2+b2));
  h = relu(BN3(h@w3+b3)); h = h@w4+b4; pooled = segment_max(h, unq_inv, V);
  pooled = where(isfinite, pooled, 0); out = relu(pooled@wc + bc)

Strategy (8 NeuronCores, SPMD):
  - Host sorts points by voxel id; device d owns voxels [d*25000,(d+1)*25000).
  - BN is shift-invariant => pre-BN biases b1/b2/b3 (and BN0's beta/mean terms)
    vanish; BN0's scale folds into w1; b4 folds into bc' = b4@wc + bc.
  - Stats passes (recompute from SBUF-resident pt_fea): pass k computes
    pre-BN y_k per 512-pt chunk via matmuls; bn_stats/bn_aggr give local
    mean/var; converted to raw moments, AllReduce'd (tiny), pad-corrected
    using the y-value of an all-zero pad point, then BN affines are built.
  - Pass 5 runs the full MLP over a host-built "round-major" point stream:
    round r holds the r-th point of each voxel (voxels in count-desc rank
    order, padded with same-voxel repeats, which are max-neutral), so
    segment-max becomes dense contiguous DVE max ops into an SBUF-resident
    pooled[256, 12544] buffer (2 phases).  Final compression matmul uses
    pooled slices directly as lhsT; empty/dummy ranks are fixed up with a
    mask to relu(bc).  Host inverse-permutes the 8 output shards.
"""

import os
import sys

sys.path.insert(0, "/opt/trn_rl_repo")

import numpy as np

# ---------------------------------------------------------------- constants
N_PTS = 600000
D_IN = 16
V_TOT = 200000
N_CORES = 8
VR = V_TOT // N_CORES          # voxels per device = 25000
RANKS = 25088                  # VR padded to multiple of 128 (196 tiles)
PHASE_R = RANKS // 2           # 12544 ranks per phase
CH = 512                       # chunk (free-dim) size
EPS = 1e-5


# ================================================================ host prep
def _host_prep(pt_fea, unq_inv):
    """Build per-device arrangements + universal (compile-time) tables."""
    unq = np.asarray(unq_inv).astype(np.int64)
    order = np.argsort(unq, kind="stable")
    sorted_unq = unq[order]
    bounds = np.searchsorted(sorted_unq, [d * VR for d in range(N_CORES + 1)])

    devs = []
    for d in range(N_CORES):
        pts_idx = order[bounds[d]:bounds[d + 1]]          # global pt indices
        loc_vox = (sorted_unq[bounds[d]:bounds[d + 1]] - d * VR).astype(np.int64)
        n_d = len(pts_idx)
        assert n_d > 0
        counts = np.bincount(loc_vox, minlength=VR)        # [VR]
        starts = np.zeros(VR + 1, np.int64)
        np.cumsum(counts, out=starts[1:])
        # count-desc rank order of the 25000 local voxels
        rank_vox = np.argsort(-counts, kind="stable")      # [VR]
        rank_cnt = counts[rank_vox]
        # pad to RANKS with dummies (vox -1, count 0)
        rank_vox_p = np.concatenate([rank_vox, -np.ones(RANKS - VR, np.int64)])
        rank_cnt_p = np.concatenate([rank_cnt, np.zeros(RANKS - VR, np.int64)])
        devs.append(dict(pts_idx=pts_idx, counts=counts, starts=starts,
                         rank_vox=rank_vox_p, rank_cnt=rank_cnt_p, n=n_d))

    # ---- universal stats-pass sizes (multiple of 8*CH so pt_st8 chunks align)
    n_max = max(dv["n"] for dv in devs)
    P_SHARD = -((n_max + CH) // -(8 * CH)) * 8 * CH
    if P_SHARD < n_max + CH:
        P_SHARD += 8 * CH                      # guarantee an all-pad last chunk
    NCH = P_SHARD // CH
    C_PAD = float(N_CORES * P_SHARD - N_PTS)

    # ---- universal round lengths per phase
    L_univ = []   # [phase][round]
    for p in range(2):
        r0, r1 = p * PHASE_R, (p + 1) * PHASE_R
        Ls = [PHASE_R]  # round 0 covers every rank (init copy)
        r = 1
        while True:
            L = max(int((dv["rank_cnt"][r0:r1] > r).sum()) for dv in devs)
            if L == 0:
                break
            Ls.append(L)
            r += 1
        L_univ.append(Ls)

    # stream offsets / lengths (per phase), padded to CH
    phase_meta = []
    for p in range(2):
        offs = np.concatenate([[0], np.cumsum(L_univ[p])]).astype(np.int64)
        sl = int(offs[-1])
        sl_pad = ((sl + CH - 1) // CH) * CH
        pad = sl_pad - sl
        phase_meta.append(dict(offs=offs, sl=sl, sl_pad=sl_pad, pad=pad))

    SL_TOT = sum(m["sl_pad"] for m in phase_meta)

    # ---- compile-time piece table per phase: chunk -> [(dst0,src0,len,is_copy)]
    pieces = []
    for p in range(2):
        m = phase_meta[p]
        offs, sl, sl_pad = m["offs"], m["sl"], m["sl_pad"]
        nrounds = len(L_univ[p])
        plist = []
        for c in range(sl_pad // CH):
            lo, hi = c * CH, (c + 1) * CH
            cps = []
            for r in range(nrounds):
                a = max(lo, int(offs[r]))
                b = min(hi, int(offs[r + 1]))
                if a < b:
                    cps.append((a - int(offs[r]), a - lo, b - a, r == 0))
            # tail pad region: maxes into ranks [(pos-sl) % PHASE_R]
            a, b = max(lo, sl), hi
            while a < b:
                dst = (a - sl) % PHASE_R
                ln = min(b - a, PHASE_R - dst)
                cps.append((dst, a - lo, ln, False))
                a += ln
            plist.append(cps)
        pieces.append(plist)

    # ---- per-device data arrays
    for dv in devs:
        pts_idx, starts, counts = dv["pts_idx"], dv["starts"], dv["counts"]
        rank_vox, rank_cnt = dv["rank_vox"], dv["rank_cnt"]
        fill0 = 0  # device-local index of first point (any valid point)

        # stats arrangement: sorted points + zero pad
        pt_sorted = pt_fea[pts_idx].astype(np.float32)            # [n,16]
        pt_pad = np.zeros((P_SHARD, D_IN), np.float32)
        pt_pad[:dv["n"]] = pt_sorted
        dv["pt_fm"] = np.ascontiguousarray(pt_pad.T)              # [16,P_SHARD]
        ps8 = P_SHARD // 8
        dv["pt_st8"] = np.ascontiguousarray(
            pt_pad.reshape(8, ps8, D_IN).transpose(0, 2, 1).reshape(128, ps8))

        # round-major stream (local point indices into pt_sorted)
        stream = []
        for p in range(2):
            r0 = p * PHASE_R
            offs = phase_meta[p]["offs"]
            for r in range(len(L_univ[p])):
                L = L_univ[p][r]
                g = rank_vox[r0:r0 + L]                   # local vox ids (-1 dummy)
                cnt = rank_cnt[r0:r0 + L]
                rr = np.minimum(r, np.maximum(cnt - 1, 0))
                idx = np.where(cnt > 0, starts[np.maximum(g, 0)] + rr, fill0)
                stream.append(idx.astype(np.int64))
            padn = phase_meta[p]["pad"]
            if padn:
                jj = np.arange(padn) % PHASE_R
                g = rank_vox[r0 + jj]
                cnt = rank_cnt[r0 + jj]
                idx = np.where(cnt > 0, starts[np.maximum(g, 0)], fill0)
                stream.append(idx.astype(np.int64))
        stream = np.concatenate(stream)
        assert len(stream) == SL_TOT
        dv["pt_rounds"] = np.ascontiguousarray(pt_sorted[stream].T)  # [16,SL_TOT]
        dv["mask"] = (rank_cnt > 0).astype(np.float32).reshape(RANKS, 1)

    tables = dict(P_SHARD=P_SHARD, NCH=NCH, C_PAD=C_PAD, L_univ=L_univ,
                  phase_meta=phase_meta, SL_TOT=SL_TOT, pieces=pieces)
    return devs, tables


# ================================================== numpy program emulation
def _emulate_device(dv, tables, params):
    """Emulate the exact device program (chunked, same op order) in numpy."""
    w1p = params["w1p"]; w2 = params["w2"]; w3 = params["w3"]; w4 = params["w4"]
    wc = params["wc"]; bcp = params["bcp"]; relu_bc = params["relu_bc"]
    sc = params["sc"]; bi = params["bi"]     # affines for bn1..bn3 (lists)

    pt_rounds = dv["pt_rounds"]              # [16, SL]
    pooled_out = np.zeros((RANKS, 16), np.float32)
    pos = 0
    for p in range(2):
        m = tables["phase_meta"][p]
        pooled = np.full((256, PHASE_R), np.nan, np.float32)
        x = pt_rounds[:, pos: pos + m["sl_pad"]]
        h1 = np.maximum((w1p.T @ x) * sc[0] + bi[0], 0)
        h2 = np.maximum((w2.T @ h1) * sc[1] + bi[1], 0)
        h3 = np.maximum((w3.T @ h2) * sc[2] + bi[2], 0)
        y4_all = w4.T @ h3                                        # [256,SLp]
        for c in range(m["sl_pad"] // CH):
            y4 = y4_all[:, c * CH:(c + 1) * CH]
            for dst0, src0, ln, is_copy in tables["pieces"][p][c]:
                seg = y4[:, src0:src0 + ln]
                if is_copy:
                    pooled[:, dst0:dst0 + ln] = seg
                else:
                    pooled[:, dst0:dst0 + ln] = np.maximum(
                        pooled[:, dst0:dst0 + ln], seg)
        pos += m["sl_pad"]
        # finalize phase
        o = pooled.T @ wc + bcp                                    # [12544,16]
        o = np.maximum(o, 0)
        msk = dv["mask"][p * PHASE_R:(p + 1) * PHASE_R]            # [12544,1]
        o = o * msk + relu_bc[None, :] * (1.0 - msk)
        pooled_out[p * PHASE_R:(p + 1) * PHASE_R] = o
    return pooled_out


def _numpy_backend(devs, tables, inputs):
    """Full numpy emulation incl. stats passes + allreduce + corrections."""
    P_SHARD, NCH, C_PAD = tables["P_SHARD"], tables["NCH"], tables["C_PAD"]
    w1 = np.asarray(inputs["w1"], np.float32)
    w2 = np.asarray(inputs["w2"], np.float32)
    w3 = np.asarray(inputs["w3"], np.float32)
    w4 = np.asarray(inputs["w4"], np.float32)
    wc = np.asarray(inputs["wc"], np.float32)
    g = [np.asarray(inputs[f"bn{k}_g"], np.float32) for k in range(4)]
    b = [np.asarray(inputs[f"bn{k}_b"], np.float32) for k in range(4)]
    bc = np.asarray(inputs["bc"], np.float32)
    bcp = np.asarray(inputs["b4"], np.float32) @ wc + bc
    relu_bc = np.maximum(bc, 0)

    # ---- pass 1: bn0 raw moments (pads are zero -> no correction)
    S = np.zeros(16, np.float64); Q = np.zeros(16, np.float64)
    for dv in devs:
        x = dv["pt_fm"]
        S += x.sum(1); Q += (x * x).sum(1)
    mean0 = (S / N_PTS).astype(np.float32)
    var0 = (Q / N_PTS).astype(np.float32) - mean0 ** 2
    c0 = g[0] / np.sqrt(var0 + EPS)
    w1p = w1 * c0[:, None]

    # ---- passes 2-4: bn1..bn3 stats with pad correction
    sc, bi = [], []
    Ws = [w1p, w2, w3]
    for k in range(3):
        S = np.zeros(Ws[k].shape[1], np.float64)
        Q = np.zeros(Ws[k].shape[1], np.float64)
        ypad = None
        for dv in devs:
            x = dv["pt_fm"]                                    # [16,P_SHARD]
            h = x
            y = None
            for j in range(k + 1):
                y = Ws[j].T @ h
                if j < k:
                    h = np.maximum(y * sc[j] + bi[j], 0)
            S += y.sum(1); Q += (y * y).sum(1)
            ypad = y[:, -1]                                    # all-pad column
        S = S - C_PAD * ypad
        Q = Q - C_PAD * ypad.astype(np.float64) ** 2
        mean = (S / N_PTS).astype(np.float32)
        var = (Q / N_PTS).astype(np.float32) - mean ** 2
        inv = 1.0 / np.sqrt(var + EPS)
        sck = g[k + 1] * inv
        sc.append(sck[:, None].astype(np.float32))
        bi.append((b[k + 1] - mean * sck)[:, None].astype(np.float32))

    params = dict(w1p=w1p, w2=w2, w3=w3, w4=w4, wc=wc, bcp=bcp,
                  relu_bc=relu_bc, sc=sc, bi=bi)
    out = np.zeros((V_TOT, 16), np.float32)
    for d, dv in enumerate(devs):
        shard = _emulate_device(dv, tables, params)            # [RANKS,16]
        rv = dv["rank_vox"][:VR]
        out[d * VR + rv] = shard[:VR]   # row g of shard <-> global rank g
    return out


# ====================================================================== API
def kernel(**inputs) -> np.ndarray:
    pt_fea = np.asarray(inputs["pt_fea"], np.float32)
    unq_inv = np.asarray(inputs["unq_inv"])
    nv = int(inputs["num_voxels"])
    assert pt_fea.shape == (N_PTS, D_IN) and nv == V_TOT

    devs, tables = _host_prep(pt_fea, unq_inv)
    backend = os.environ.get("CYL_BACKEND", "bass")
    if backend == "numpy":
        return _numpy_backend(devs, tables, inputs)
    return _bass_backend(devs, tables, inputs)


# ============================================================= bass backend
def _build_program(tables):
    import concourse.bass as bass
    import concourse.bacc as bacc
    import concourse.mybir as mybir
    import concourse.tile as tile
    from contextlib import ExitStack

    P_SHARD, NCH, C_PAD = tables["P_SHARD"], tables["NCH"], tables["C_PAD"]
    SL_TOT = tables["SL_TOT"]
    PS8 = P_SHARD // 8
    F32 = mybir.dt.float32
    AF = mybir.ActivationFunctionType
    OP = mybir.AluOpType
    INV_N = 1.0 / float(N_PTS)

    nc = bacc.Bacc("TRN2", target_bir_lowering=False, debug=False,
                   num_devices=N_CORES)

    # ---------------- DRAM I/O
    d_pt8 = nc.dram_tensor("pt_st8", [128, PS8], F32, kind="ExternalInput")
    d_ptfm = nc.dram_tensor("pt_fm", [16, P_SHARD], F32, kind="ExternalInput")
    d_ptr = nc.dram_tensor("pt_rounds", [16, SL_TOT], F32, kind="ExternalInput")
    d_mask = nc.dram_tensor("mask", [RANKS, 1], F32, kind="ExternalInput")
    d_w1 = nc.dram_tensor("w1", [16, 64], F32, kind="ExternalInput")
    d_w2 = nc.dram_tensor("w2", [64, 128], F32, kind="ExternalInput")
    d_w3 = nc.dram_tensor("w3", [128, 256], F32, kind="ExternalInput")
    d_w4 = nc.dram_tensor("w4", [256, 256], F32, kind="ExternalInput")
    d_wc = nc.dram_tensor("wc", [256, 16], F32, kind="ExternalInput")
    d_g0 = nc.dram_tensor("g0", [16, 1], F32, kind="ExternalInput")
    d_g1 = nc.dram_tensor("g1", [64, 1], F32, kind="ExternalInput")
    d_b1 = nc.dram_tensor("b1", [64, 1], F32, kind="ExternalInput")
    d_g2 = nc.dram_tensor("g2", [128, 1], F32, kind="ExternalInput")
    d_b2 = nc.dram_tensor("b2", [128, 1], F32, kind="ExternalInput")
    d_g3 = nc.dram_tensor("g3", [256, 1], F32, kind="ExternalInput")
    d_b3 = nc.dram_tensor("b3", [256, 1], F32, kind="ExternalInput")
    d_bcp = nc.dram_tensor("bcp_b", [128, 16], F32, kind="ExternalInput")
    d_rbc = nc.dram_tensor("rbc_b", [128, 16], F32, kind="ExternalInput")
    d_sel = nc.dram_tensor("sel", [128, 16], F32, kind="ExternalInput")
    d_out = nc.dram_tensor("out", [RANKS, 16], F32, kind="ExternalOutput")

    cc_in, cc_out = [], []
    for k, D in enumerate([16, 64, 128, 128, 128]):
        cc_in.append(nc.dram_tensor(f"cc_in{k}", [D, 2], F32))
        cc_out.append(nc.dram_tensor(f"cc_out{k}", [D, 2], F32,
                                     addr_space="Shared"))
    rg = [list(range(N_CORES))]

    with ExitStack() as ctx:
        tc = ctx.enter_context(tile.TileContext(nc))
        cpool = ctx.enter_context(tc.tile_pool(name="const", bufs=1))
        spool = ctx.enter_context(tc.tile_pool(name="small", bufs=1))
        xpool = ctx.enter_context(tc.tile_pool(name="xin", bufs=4))
        hpool = ctx.enter_context(tc.tile_pool(name="act", bufs=3))
        ppool1 = ctx.enter_context(tc.tile_pool(name="ps1", bufs=1, space="PSUM"))
        ppool2 = ctx.enter_context(tc.tile_pool(name="ps2", bufs=1, space="PSUM"))
        ppool3 = ctx.enter_context(tc.tile_pool(name="ps3", bufs=1, space="PSUM"))
        ppool4 = ctx.enter_context(tc.tile_pool(name="ps4", bufs=2, space="PSUM"))

        # ---------------- constants
        def load(pool, dram, shape, tag):
            t = pool.tile(shape, F32, tag=tag)
            nc.sync.dma_start(out=t[:], in_=dram.ap())
            return t

        w1t = load(cpool, d_w1, [16, 64], "w1")
        w2t = load(cpool, d_w2, [64, 128], "w2")
        w3t = load(cpool, d_w3, [128, 256], "w3")
        w4t = cpool.tile([128, 512], F32, tag="w4")   # [k_half][m 0:256|256:512]
        nc.sync.dma_start(out=w4t[:, 0:256], in_=d_w4.ap()[0:128, :])
        nc.sync.dma_start(out=w4t[:, 256:512], in_=d_w4.ap()[128:256, :])
        wct = cpool.tile([128, 32], F32, tag="wc")    # [k_half][16 cols each]
        nc.sync.dma_start(out=wct[:, 0:16], in_=d_wc.ap()[0:128, :])
        nc.sync.dma_start(out=wct[:, 16:32], in_=d_wc.ap()[128:256, :])
        g0t = load(cpool, d_g0, [16, 1], "g0")
        g1t = load(cpool, d_g1, [64, 1], "g1")
        b1t = load(cpool, d_b1, [64, 1], "b1")
        g2t = load(cpool, d_g2, [128, 1], "g2")
        b2t = load(cpool, d_b2, [128, 1], "b2")
        g3t = cpool.tile([128, 2], F32, tag="g3")
        nc.sync.dma_start(out=g3t[:, 0:1], in_=d_g3.ap()[0:128, :])
        nc.sync.dma_start(out=g3t[:, 1:2], in_=d_g3.ap()[128:256, :])
        b3t = cpool.tile([128, 2], F32, tag="b3")
        nc.sync.dma_start(out=b3t[:, 0:1], in_=d_b3.ap()[0:128, :])
        nc.sync.dma_start(out=b3t[:, 1:2], in_=d_b3.ap()[128:256, :])
        bcpt = load(cpool, d_bcp, [128, 16], "bcp")
        rbct = load(cpool, d_rbc, [128, 16], "rbc")
        selt = load(cpool, d_sel, [128, 16], "sel")

        w1p = cpool.tile([16, 64], F32, tag="w1p")
        sc1 = cpool.tile([64, 1], F32, tag="sc1")
        bi1 = cpool.tile([64, 1], F32, tag="bi1")
        sc2 = cpool.tile([128, 1], F32, tag="sc2")
        bi2 = cpool.tile([128, 1], F32, tag="bi2")
        sc3 = cpool.tile([128, 2], F32, tag="sc3")    # two halves
        bi3 = cpool.tile([128, 2], F32, tag="bi3")

        # small scratch for affine math
        def scratch(D, tag):
            return spool.tile([D, 1], F32, tag=tag, name=tag)

        # ---- helper: from aggregated (mean,var) [D,2] + optional ypad [D,1]
        #      to (sc, bi) via allreduce; g/b None -> returns c0-style scale only
        def affine_from_stats(k, agg, ypad, gt, bt, sct, bit, D):
            # convert to raw sums: S = PS*mean ; Q = PS*var + PS*mean^2
            mn = agg[:, 0:1]
            vr = agg[:, 1:2]
            msq = scratch(D, f"msq{k}")
            nc.vector.tensor_tensor(out=msq[:], in0=mn, in1=mn, op=OP.mult)
            pk = spool.tile([D, 2], F32, tag=f"pk{k}")
            nc.vector.tensor_scalar_mul(out=pk[:, 0:1], in0=mn,
                                        scalar1=float(P_SHARD))
            t2 = scratch(D, f"t2{k}")
            nc.vector.tensor_tensor(out=t2[:], in0=vr, in1=msq[:], op=OP.add)
            nc.vector.tensor_scalar_mul(out=pk[:, 1:2], in0=t2[:],
                                        scalar1=float(P_SHARD))
            nc.sync.dma_start(out=cc_in[k].ap(), in_=pk[:])
            nc.gpsimd.collective_compute(
                "AllReduce", OP.add, replica_groups=rg,
                ins=[cc_in[k].ap().opt()], outs=[cc_out[k].ap().opt()])
            ar = spool.tile([D, 2], F32, tag=f"ar{k}")
            nc.sync.dma_start(out=ar[:], in_=cc_out[k].ap())
            S = scratch(D, f"S{k}")
            Q = scratch(D, f"Q{k}")
            if ypad is not None:
                tmp = scratch(D, f"yc{k}")
                nc.vector.tensor_scalar_mul(out=tmp[:], in0=ypad[:],
                                            scalar1=-C_PAD)
                nc.vector.tensor_tensor(out=S[:], in0=ar[:, 0:1], in1=tmp[:],
                                        op=OP.add)
                ysq = scratch(D, f"ysq{k}")
                nc.vector.tensor_tensor(out=ysq[:], in0=ypad[:], in1=ypad[:],
                                        op=OP.mult)
                nc.vector.tensor_scalar_mul(out=tmp[:], in0=ysq[:],
                                            scalar1=-C_PAD)
                nc.vector.tensor_tensor(out=Q[:], in0=ar[:, 1:2], in1=tmp[:],
                                        op=OP.add)
            else:
                nc.vector.tensor_copy(out=S[:], in_=ar[:, 0:1])
                nc.vector.tensor_copy(out=Q[:], in_=ar[:, 1:2])
            mean = scratch(D, f"mean{k}")
            nc.vector.tensor_scalar_mul(out=mean[:], in0=S[:], scalar1=INV_N)
            ex2 = scratch(D, f"ex2{k}")
            nc.vector.tensor_scalar_mul(out=ex2[:], in0=Q[:], scalar1=INV_N)
            m2 = scratch(D, f"m2{k}")
            nc.vector.tensor_tensor(out=m2[:], in0=mean[:], in1=mean[:],
                                    op=OP.mult)
            var = scratch(D, f"var{k}")
            nc.vector.tensor_tensor(out=var[:], in0=ex2[:], in1=m2[:],
                                    op=OP.subtract)
            nc.vector.tensor_scalar_add(out=var[:], in0=var[:], scalar1=EPS)
            sd = scratch(D, f"sd{k}")
            nc.scalar.activation(out=sd[:], in_=var[:], func=AF.Sqrt, bias=0.0)
            inv = scratch(D, f"inv{k}")
            nc.vector.reciprocal(out=inv[:], in_=sd[:])
            nc.vector.tensor_tensor(out=sct, in0=gt, in1=inv[:], op=OP.mult)
            if bit is not None:
                t3 = scratch(D, f"t3{k}")
                nc.vector.tensor_tensor(out=t3[:], in0=mean[:], in1=sct,
                                        op=OP.mult)
                nc.vector.tensor_tensor(out=bit, in0=bt, in1=t3[:],
                                        op=OP.subtract)

        # ================= pass 1: bn0 stats from pt_st8
        with tc.tile_pool(name="p1pool", bufs=1) as p1pool:
            pt8 = p1pool.tile([128, PS8], F32, tag="pt8")
            nc.sync.dma_start(out=pt8[:], in_=d_pt8.ap())
            n1 = -(PS8 // -CH)
            st0 = p1pool.tile([128, n1 * 6], F32, tag="st0")
            for i in range(n1):
                a, b = i * CH, min((i + 1) * CH, PS8)
                nc.vector.bn_stats(out=st0[:, i * 6:(i + 1) * 6],
                                   in_=pt8[:, a:b])
            agg0 = p1pool.tile([128, 2], F32, tag="agg0")
            nc.vector.bn_aggr(out=agg0[:], in_=st0[:])
            # -> per (group,feature) sums; combine groups via sel matmul
            mn = agg0[:, 0:1]
            vr = agg0[:, 1:2]
            packed = p1pool.tile([128, 2], F32, tag="pkd0")
            nc.vector.tensor_scalar_mul(out=packed[:, 0:1], in0=mn,
                                        scalar1=float(PS8))
            msq = p1pool.tile([128, 1], F32, tag="msq0")
            nc.vector.tensor_tensor(out=msq[:], in0=mn, in1=mn, op=OP.mult)
            t2 = p1pool.tile([128, 1], F32, tag="t20")
            nc.vector.tensor_tensor(out=t2[:], in0=vr, in1=msq[:], op=OP.add)
            nc.vector.tensor_scalar_mul(out=packed[:, 1:2], in0=t2[:],
                                        scalar1=float(PS8))
            ps0 = ppool1.tile([16, 2], F32, tag="p1")
            nc.tensor.matmul(out=ps0[:], lhsT=selt[:], rhs=packed[:],
                             start=True, stop=True)
            pk0 = p1pool.tile([16, 2], F32, tag="pk0s")
            nc.vector.tensor_copy(out=pk0[:], in_=ps0[:])
            nc.sync.dma_start(out=cc_in[0].ap(), in_=pk0[:])
            nc.gpsimd.collective_compute(
                "AllReduce", OP.add, replica_groups=rg,
                ins=[cc_in[0].ap().opt()], outs=[cc_out[0].ap().opt()])
            ar0 = spool.tile([16, 2], F32, tag="ar0")
            nc.sync.dma_start(out=ar0[:], in_=cc_out[0].ap())
            # c0 = g0 / sqrt(var0+eps)
            mean0 = spool.tile([16, 1], F32, tag="mean0")
            nc.vector.tensor_scalar_mul(out=mean0[:], in0=ar0[:, 0:1],
                                        scalar1=INV_N)
            ex20 = spool.tile([16, 1], F32, tag="ex20")
            nc.vector.tensor_scalar_mul(out=ex20[:], in0=ar0[:, 1:2],
                                        scalar1=INV_N)
            m20 = spool.tile([16, 1], F32, tag="m20")
            nc.vector.tensor_tensor(out=m20[:], in0=mean0[:], in1=mean0[:],
                                    op=OP.mult)
            var0 = spool.tile([16, 1], F32, tag="var0")
            nc.vector.tensor_tensor(out=var0[:], in0=ex20[:], in1=m20[:],
                                    op=OP.subtract)
            nc.vector.tensor_scalar_add(out=var0[:], in0=var0[:],
                                        scalar1=EPS)
            sd0 = spool.tile([16, 1], F32, tag="sd0")
            nc.scalar.activation(out=sd0[:], in_=var0[:], func=AF.Sqrt,
                                 bias=0.0)
            inv0 = spool.tile([16, 1], F32, tag="inv0")
            nc.vector.reciprocal(out=inv0[:], in_=sd0[:])
            c0 = spool.tile([16, 1], F32, tag="c0")
            nc.vector.tensor_tensor(out=c0[:], in0=g0t[:], in1=inv0[:],
                                    op=OP.mult)
            nc.vector.tensor_scalar_mul(out=w1p[:], in0=w1t[:],
                                        scalar1=c0[:, 0:1])

        # ================= passes 2-4: bn1..bn3 stats
        def mlp_chunk(rhs_ap, depth, c=None, sbufs=None, ypads=None):
            """Compute through `depth` linear layers; record stats at depth."""
            p1 = ppool1.tile([64, CH], F32, tag="p1")
            nc.tensor.matmul(out=p1[:], lhsT=w1p[:], rhs=rhs_ap,
                             start=True, stop=True)
            if depth == 1:
                nc.vector.bn_stats(out=sbufs[0][:, c * 6:(c + 1) * 6],
                                   in_=p1[:])
                return None
            h1 = hpool.tile([64, CH], F32, tag="h1")
            nc.scalar.activation(out=h1[:], in_=p1[:], func=AF.Relu,
                                 scale=sc1[:, 0:1], bias=bi1[:, 0:1])
            p2 = ppool2.tile([128, CH], F32, tag="p2")
            nc.tensor.matmul(out=p2[:], lhsT=w2t[:], rhs=h1[:],
                             start=True, stop=True)
            if depth == 2:
                nc.vector.bn_stats(out=sbufs[0][:, c * 6:(c + 1) * 6],
                                   in_=p2[:])
                if ypads is not None:
                    nc.vector.tensor_copy(out=ypads[0][:],
                                          in_=p2[:, CH - 1:CH])
                return None
            h2 = hpool.tile([128, CH], F32, tag="h2")
            nc.scalar.activation(out=h2[:], in_=p2[:], func=AF.Relu,
                                 scale=sc2[:, 0:1], bias=bi2[:, 0:1])
            p3a = ppool3.tile([128, CH], F32, tag="p3a")
            nc.tensor.matmul(out=p3a[:], lhsT=w3t[:, 0:128], rhs=h2[:],
                             start=True, stop=True)
            p3b = ppool3.tile([128, CH], F32, tag="p3b")
            nc.tensor.matmul(out=p3b[:], lhsT=w3t[:, 128:256], rhs=h2[:],
                             start=True, stop=True)
            if depth == 3:
                nc.vector.bn_stats(out=sbufs[0][:, c * 6:(c + 1) * 6],
                                   in_=p3a[:])
                nc.vector.bn_stats(out=sbufs[1][:, c * 6:(c + 1) * 6],
                                   in_=p3b[:])
                if ypads is not None:
                    nc.vector.tensor_copy(out=ypads[0][:],
                                          in_=p3a[:, CH - 1:CH])
                    nc.vector.tensor_copy(out=ypads[1][:],
                                          in_=p3b[:, CH - 1:CH])
                return None
            # depth 4: full chain to y4 psum pair
            h3a = hpool.tile([128, CH], F32, tag="h3a")
            nc.scalar.activation(out=h3a[:], in_=p3a[:], func=AF.Relu,
                                 scale=sc3[:, 0:1], bias=bi3[:, 0:1])
            h3b = hpool.tile([128, CH], F32, tag="h3b")
            nc.scalar.activation(out=h3b[:], in_=p3b[:], func=AF.Relu,
                                 scale=sc3[:, 1:2], bias=bi3[:, 1:2])
            p4a = ppool4.tile([128, CH], F32, tag="p4a")
            nc.tensor.matmul(out=p4a[:], lhsT=w4t[:, 0:128], rhs=h3a[:],
                             start=True, stop=False)
            nc.tensor.matmul(out=p4a[:], lhsT=w4t[:, 256:384], rhs=h3b[:],
                             start=False, stop=True)
            p4b = ppool4.tile([128, CH], F32, tag="p4b")
            nc.tensor.matmul(out=p4b[:], lhsT=w4t[:, 128:256], rhs=h3a[:],
                             start=True, stop=False)
            nc.tensor.matmul(out=p4b[:], lhsT=w4t[:, 384:512], rhs=h3b[:],
                             start=False, stop=True)
            return p4a, p4b

        def load_chunk(dram, c):
            x = xpool.tile([16, CH], F32, tag="x")
            nc.sync.dma_start(out=x[:], in_=dram.ap()[:, c * CH:(c + 1) * CH])
            return x

        # pass 2 (bn1): y1 pad value is exactly 0 -> no correction
        sb1 = cpool.tile([64, NCH * 6], F32, tag="sb1")
        for c in range(NCH):
            x = load_chunk(d_ptfm, c)
            mlp_chunk(x[:], 1, c, [sb1])
        agg1 = spool.tile([64, 2], F32, tag="agg1")
        nc.vector.bn_aggr(out=agg1[:], in_=sb1[:])
        affine_from_stats(1, agg1, None, g1t[:], b1t[:], sc1[:], bi1[:], 64)

        # pass 3 (bn2)
        sb2 = cpool.tile([128, NCH * 6], F32, tag="sb2")
        yp2 = spool.tile([128, 1], F32, tag="yp2")
        for c in range(NCH):
            x = load_chunk(d_ptfm, c)
            mlp_chunk(x[:], 2, c, [sb2], [yp2] if c == NCH - 1 else None)
        agg2 = spool.tile([128, 2], F32, tag="agg2")
        nc.vector.bn_aggr(out=agg2[:], in_=sb2[:])
        affine_from_stats(2, agg2, yp2, g2t[:], b2t[:], sc2[:], bi2[:], 128)

        # pass 4 (bn3) - two halves packed as [128, 2]
        sb3a = cpool.tile([128, NCH * 6], F32, tag="sb3a")
        sb3b = cpool.tile([128, NCH * 6], F32, tag="sb3b")
        yp3a = spool.tile([128, 1], F32, tag="yp3a")
        yp3b = spool.tile([128, 1], F32, tag="yp3b")
        for c in range(NCH):
            x = load_chunk(d_ptfm, c)
            mlp_chunk(x[:], 3, c, [sb3a, sb3b],
                      [yp3a, yp3b] if c == NCH - 1 else None)
        agg3a = spool.tile([128, 2], F32, tag="agg3a")
        nc.vector.bn_aggr(out=agg3a[:], in_=sb3a[:])
        affine_from_stats(3, agg3a, yp3a, g3t[:, 0:1], b3t[:, 0:1],
                          sc3[:, 0:1], bi3[:, 0:1], 128)
        agg3b = spool.tile([128, 2], F32, tag="agg3b")
        nc.vector.bn_aggr(out=agg3b[:], in_=sb3b[:])
        affine_from_stats(4, agg3b, yp3b, g3t[:, 1:2], b3t[:, 1:2],
                          sc3[:, 1:2], bi3[:, 1:2], 128)

        # ================= pass 5: round-major max-pool + compression
        with tc.tile_pool(name="pooled", bufs=1) as plpool, \
             tc.tile_pool(name="fin", bufs=3) as fpool:
            base = 0
            for p in range(2):
                m = tables["phase_meta"][p]
                pooled_a = plpool.tile([128, PHASE_R], F32, tag="pa")
                pooled_b = plpool.tile([128, PHASE_R], F32, tag="pb")
                for c in range(m["sl_pad"] // CH):
                    x = xpool.tile([16, CH], F32, tag="x")
                    a0 = base + c * CH
                    nc.sync.dma_start(out=x[:],
                                      in_=d_ptr.ap()[:, a0:a0 + CH])
                    p4a, p4b = mlp_chunk(x[:], 4)
                    for dst0, src0, ln, is_copy in tables["pieces"][p][c]:
                        for pooled, p4 in ((pooled_a, p4a), (pooled_b, p4b)):
                            if is_copy:
                                nc.vector.tensor_copy(
                                    out=pooled[:, dst0:dst0 + ln],
                                    in_=p4[:, src0:src0 + ln])
                            else:
                                nc.vector.tensor_tensor(
                                    out=pooled[:, dst0:dst0 + ln],
                                    in0=pooled[:, dst0:dst0 + ln],
                                    in1=p4[:, src0:src0 + ln], op=OP.max)
                base += m["sl_pad"]
                # finalize this phase
                for j in range(PHASE_R // 128):
                    po = ppool4.tile([128, 16], F32,
                                     tag="p4a" if j % 2 == 0 else "p4b")
                    nc.tensor.matmul(out=po[:],
                                     lhsT=pooled_a[:, j * 128:(j + 1) * 128],
                                     rhs=wct[:, 0:16], start=True, stop=False)
                    nc.tensor.matmul(out=po[:],
                                     lhsT=pooled_b[:, j * 128:(j + 1) * 128],
                                     rhs=wct[:, 16:32], start=False, stop=True)
                    mt = fpool.tile([128, 1], F32, tag="mt")
                    row = p * PHASE_R + j * 128
                    nc.sync.dma_start(out=mt[:],
                                      in_=d_mask.ap()[row:row + 128, :])
                    o1 = fpool.tile([128, 16], F32, tag="o1")
                    nc.vector.tensor_tensor(out=o1[:], in0=po[:], in1=bcpt[:],
                                            op=OP.add)
                    nc.vector.tensor_scalar_max(out=o1[:], in0=o1[:],
                                                scalar1=0.0)
                    nc.vector.tensor_scalar_mul(out=o1[:], in0=o1[:],
                                                scalar1=mt[:, 0:1])
                    em = fpool.tile([128, 1], F32, tag="em")
                    nc.vector.tensor_scalar(out=em[:], in0=mt[:], scalar1=-1.0,
                                            scalar2=1.0, op0=OP.mult,
                                            op1=OP.add)
                    e1 = fpool.tile([128, 16], F32, tag="e1")
                    nc.vector.tensor_scalar_mul(out=e1[:], in0=rbct[:],
                                                scalar1=em[:, 0:1])
                    nc.vector.tensor_tensor(out=o1[:], in0=o1[:], in1=e1[:],
                                            op=OP.add)
                    nc.sync.dma_start(out=d_out.ap()[row:row + 128, :],
                                      in_=o1[:])

    nc.compile()
    return nc


def _bench(nc, in_maps, nbench):
    """Time repeated warm executions of the compiled program (dev only)."""
    import time
    import jax
    import numpy as np_
    from jax.sharding import Mesh, PartitionSpec
    from jax.experimental.shard_map import shard_map
    import concourse.mybir as mybir
    from concourse import bass2jax

    bass2jax.install_neuronx_cc_hook()
    pname = nc.partition_id_tensor.name if nc.partition_id_tensor else None
    in_names, out_names, out_avals, zero_outs = [], [], [], []
    for alloc in nc.m.functions[0].allocations:
        if not isinstance(alloc, mybir.MemoryLocationSet):
            continue
        name = alloc.memorylocations[0].name
        if alloc.kind == "ExternalInput":
            if name != pname:
                in_names.append(name)
        elif alloc.kind == "ExternalOutput":
            shape = tuple(alloc.tensor_shape)
            dtype = mybir.dt.np(alloc.dtype)
            out_names.append(name)
            out_avals.append(jax.core.ShapedArray(shape, dtype))
            zero_outs.append(np_.zeros(shape, dtype))
    n_params = len(in_names)
    all_names = in_names + out_names
    if pname is not None:
        all_names = all_names + [pname]

    def _body(*args):
        operands = list(args)
        if pname is not None:
            operands.append(bass2jax.partition_id_tensor())
        outs = bass2jax._bass_exec_p.bind(
            *operands, out_avals=tuple(out_avals), in_names=tuple(all_names),
            out_names=tuple(out_names), lowering_input_output_aliases=(),
            sim_require_finite=True, sim_require_nnan=True, nc=nc)
        return tuple(outs)

    devices = jax.devices()[:N_CORES]
    mesh = Mesh(np_.asarray(devices), ("core",))
    n_outs = len(out_names)
    sharded = jax.jit(
        shard_map(_body, mesh=mesh,
                  in_specs=(PartitionSpec("core"),) * (n_params + n_outs),
                  out_specs=(PartitionSpec("core"),) * n_outs,
                  check_rep=False),
        keep_unused=True)
    concat_in = [np_.concatenate([np_.asarray(m[n]) for m in in_maps], axis=0)
                 for n in in_names]
    concat_zeros = [np_.zeros((N_CORES * z.shape[0], *z.shape[1:]), z.dtype)
                    for z in zero_outs]
    sh_in = jax.device_put(
        concat_in + concat_zeros,
        [jax.sharding.NamedSharding(mesh, PartitionSpec("core"))]
        * (n_params + n_outs))
    out = sharded(*sh_in)
    jax.block_until_ready(out)
    times = []
    for _ in range(nbench):
        t0 = time.perf_counter()
        out = sharded(*sh_in)
        jax.block_until_ready(out)
        times.append(time.perf_counter() - t0)
    times = np_.array(times) * 1e9
    print(f"HW exec time: {int(times.min())} ns")
    print(f"bench wall ns: min {times.min():.0f} med {np_.median(times):.0f} "
          f"mean {times.mean():.0f} n={nbench}")


def _bass_backend(devs, tables, inputs):
    from concourse import bass_utils

    nc = _build_program(tables)

    w1 = np.ascontiguousarray(np.asarray(inputs["w1"], np.float32))
    w2 = np.ascontiguousarray(np.asarray(inputs["w2"], np.float32))
    w3 = np.ascontiguousarray(np.asarray(inputs["w3"], np.float32))
    w4 = np.ascontiguousarray(np.asarray(inputs["w4"], np.float32))
    wc = np.ascontiguousarray(np.asarray(inputs["wc"], np.float32))
    bc = np.asarray(inputs["bc"], np.float32)
    b4 = np.asarray(inputs["b4"], np.float32)
    bcp = b4 @ wc + bc
    relu_bc = np.maximum(bc, 0)
    sel = (np.arange(128)[:, None] % 16 == np.arange(16)[None, :]).astype(
        np.float32)

    shared = {
        "w1": w1, "w2": w2, "w3": w3, "w4": w4, "wc": wc,
        "g0": np.asarray(inputs["bn0_g"], np.float32).reshape(16, 1),
        "g1": np.asarray(inputs["bn1_g"], np.float32).reshape(64, 1),
        "b1": np.asarray(inputs["bn1_b"], np.float32).reshape(64, 1),
        "g2": np.asarray(inputs["bn2_g"], np.float32).reshape(128, 1),
        "b2": np.asarray(inputs["bn2_b"], np.float32).reshape(128, 1),
        "g3": np.asarray(inputs["bn3_g"], np.float32).reshape(256, 1),
        "b3": np.asarray(inputs["bn3_b"], np.float32).reshape(256, 1),
        "bcp_b": np.tile(bcp.reshape(1, 16), (128, 1)).astype(np.float32),
        "rbc_b": np.tile(relu_bc.reshape(1, 16), (128, 1)).astype(np.float32),
        "sel": sel,
    }
    in_maps = []
    for dv in devs:
        im = dict(shared)
        im["pt_st8"] = dv["pt_st8"]
        im["pt_fm"] = dv["pt_fm"]
        im["pt_rounds"] = dv["pt_rounds"]
        im["mask"] = dv["mask"]
        in_maps.append(im)

    nbench = int(os.environ.get("CYL_BENCH", "0"))
    if nbench:
        _bench(nc, in_maps, nbench)
    res = bass_utils.run_bass_kernel_spmd(nc, in_maps, list(range(N_CORES)))

    out = np.zeros((V_TOT, 16), np.float32)
    for d, dv in enumerate(devs):
        shard = res.results[d]["out"]
        rv = dv["rank_vox"][:VR]
        out[d * VR + rv] = shard[:VR]
    return out


if __name__ == "__main__":
    pass
